# revision 2
# baseline (speedup 1.0000x reference)
"""Distributed single-head attention on 8 TRN2 NeuronCores.

softmax(Q @ K.T / sqrt(128)) @ V  with Q,K,V: [8192, 128] fp32.

Strategy: query-parallel. Q rows are sharded 8 ways (1024 queries/core);
K and V are replicated. Each core runs flash-attention-style in the
"S^T" layout (partitions = keys) so the PV matmul needs no transpose of
the probability tiles:

  S^T[k, q] = (KT_tile).T @ QT          (KT tile stationary, QT moving)
  P^T       = exp(S^T / sqrt(128))      (ACT, fused scale; no max-sub
                                         needed: |scores| <= ~6 in fp32)
  O^T[d, q] += (V_tile).T @ P^T         (V tile is [keys, d] in DRAM =
                                         already the stationary layout)
  l[q]      = ones.T @ sum_k_tiles(P^T) (PSUM-broadcast row sums)
  O         = transpose(O^T * (1/l))

Matmuls run in bf16 (fp32 matmul is 4 cyc/row on TRN2; bf16 is 1),
accumulation stays fp32 in PSUM; the softmax denominator accumulates in
bf16 but its rounding error averages down over the final 128-partition
reduction (~0.15% on l).
"""

import sys

try:
    import concourse  # noqa: F401
except ImportError:  # grading container fallback
    sys.path.insert(0, "/opt/trn_rl_repo")

import numpy as np

import concourse.bass as bass
import concourse.tile as tile
from concourse import bacc, mybir
from concourse.bass_utils import run_bass_kernel_spmd
from concourse.masks import make_identity

N_CORES = 8
NQ, NK, D = 8192, 8192, 128
NQS = NQ // N_CORES          # queries per core
KT_TILES = NK // 128         # 64 key tiles of 128
QCHUNK = 512                 # matmul moving-dim size (1 PSUM bank fp32)
N_QCHUNK = NQS // QCHUNK     # 2
G = 2                        # key tiles exp'd per ACT instruction
SCALE = 1.0 / np.sqrt(np.float32(D))

F32 = mybir.dt.float32
BF16 = mybir.dt.bfloat16

_COMPILED = None


def _build():
    nc = bacc.Bacc(
        "TRN2", target_bir_lowering=False, debug=False, num_devices=N_CORES
    )
    q_d = nc.dram_tensor("Q", [NQS, D], F32, kind="ExternalInput").ap()
    k_d = nc.dram_tensor("K", [NK, D], F32, kind="ExternalInput").ap()
    v_d = nc.dram_tensor("V", [NK, D], F32, kind="ExternalInput").ap()
    o_d = nc.dram_tensor("out", [NQS, D], F32, kind="ExternalOutput").ap()

    # tile views: row = a*128 + p
    q_r = q_d.rearrange("(a p) d -> p a d", p=128)   # [128, 8, 128]
    k_r = k_d.rearrange("(a p) d -> p a d", p=128)   # [128, 64, 128]
    v_r = v_d.rearrange("(a p) d -> p a d", p=128)
    o_r = o_d.rearrange("(a p) d -> p a d", p=128)   # [128, 8, 128]

    with tile.TileContext(nc) as tc:
        with (
            tc.tile_pool(name="persist", bufs=1) as persist,
            tc.tile_pool(name="stage", bufs=3) as stage,
            tc.tile_pool(name="pt", bufs=3) as ptp,
            tc.tile_pool(name="psum_s", bufs=2, space="PSUM") as psum_s,
            tc.tile_pool(name="psum_o", bufs=1, space="PSUM") as psum_o,
            tc.tile_pool(name="psum_tr", bufs=2, space="PSUM") as psum_tr,
        ):
            ident = persist.tile([128, 128], F32)
            make_identity(nc, ident)
            ones_b = persist.tile([128, 128], BF16)
            nc.gpsimd.memset(ones_b, 1.0)

            kt_sb = persist.tile([128, NK], BF16)      # K^T  [d, keys]
            v_sb = persist.tile([128, KT_TILES, D], BF16)  # [keys(128/t), t, d]
            qt_sb = persist.tile([128, NQS], BF16)     # Q^T  [d, q]
            acc = persist.tile([128, NQS], BF16)       # sum_t P^T tiles
            rl = persist.tile([128, NQS], F32)         # 1/l bcast over parts
            otn = persist.tile([128, NQS], F32)        # O^T / l
            out_sb = persist.tile([128, NQS // 128, D], F32)

            nc.gpsimd.memset(acc, 0.0)

            # ---- load + transpose K (64 tiles), Q (8 tiles); convert V ----
            # staging: 8 key/query tiles per DMA
            for i in range(8):  # K: 8 stages x 8 tiles
                st = stage.tile([128, 8, 128], F32, tag="stage")
                nc.sync.dma_start(out=st, in_=k_r[:, 8 * i : 8 * i + 8, :])
                for h in range(2):  # 2 psum banks of 4 transposes
                    ps = psum_tr.tile([128, 512], F32, tag="tr")
                    for j in range(4):
                        nc.tensor.transpose(
                            ps[:, 128 * j : 128 * (j + 1)],
                            st[:, 4 * h + j, :],
                            ident,
                        )
                    c0 = (8 * i + 4 * h) * 128
                    nc.vector.tensor_copy(out=kt_sb[:, c0 : c0 + 512], in_=ps)
            for i in range(2):  # Q: 2 stages x 4 tiles
                st = stage.tile([128, 4, 128], F32, tag="qstage")
                nc.sync.dma_start(out=st, in_=q_r[:, 4 * i : 4 * i + 4, :])
                ps = psum_tr.tile([128, 512], F32, tag="tr")
                for j in range(4):
                    nc.tensor.transpose(
                        ps[:, 128 * j : 128 * (j + 1)], st[:, j, :], ident
                    )
                nc.vector.tensor_copy(
                    out=qt_sb[:, 512 * i : 512 * (i + 1)], in_=ps
                )
            for i in range(8):  # V: straight convert, no transpose
                st = stage.tile([128, 8, 128], F32, tag="stage")
                nc.sync.dma_start(out=st, in_=v_r[:, 8 * i : 8 * i + 8, :])
                nc.vector.tensor_copy(out=v_sb[:, 8 * i : 8 * i + 8, :], in_=st)

            # ---- main loop ----
            po = psum_o.tile([128, NQS], F32)  # O^T accum, both chunks
            for c in range(N_QCHUNK):
                q0 = c * QCHUNK
                qs = slice(q0, q0 + QCHUNK)
                for g in range(KT_TILES // G):
                    ps = psum_s.tile([128, G * QCHUNK], F32)
                    for j in range(G):
                        t = G * g + j
                        nc.tensor.matmul(
                            ps[:, QCHUNK * j : QCHUNK * (j + 1)],
                            kt_sb[:, 128 * t : 128 * (t + 1)],
                            qt_sb[:, qs],
                            start=True,
                            stop=True,
                        )
                    pt = ptp.tile([128, G * QCHUNK], BF16, tag="pt")
                    nc.scalar.activation(
                        pt, ps, mybir.ActivationFunctionType.Exp, scale=float(SCALE)
                    )
                    for j in range(G):
                        t = G * g + j
                        pj = pt[:, QCHUNK * j : QCHUNK * (j + 1)]
                        nc.vector.tensor_add(acc[:, qs], acc[:, qs], pj)
                        nc.tensor.matmul(
                            po[:, qs],
                            v_sb[:, t, :],
                            pj,
                            start=(t == 0),
                            stop=(t == KT_TILES - 1),
                        )

                # ---- epilogue for this chunk ----
                pl = psum_tr.tile([128, QCHUNK], F32, tag="tr")
                nc.tensor.matmul(pl, ones_b, acc[:, qs], start=True, stop=True)
                nc.vector.reciprocal(rl[:, qs], pl)
                nc.vector.tensor_mul(otn[:, qs], po[:, qs], rl[:, qs])
                pso = psum_tr.tile([128, QCHUNK], F32, tag="tr")
                for j in range(4):
                    nc.tensor.transpose(
                        pso[:, 128 * j : 128 * (j + 1)],
                        otn[:, q0 + 128 * j : q0 + 128 * (j + 1)],
                        ident,
                    )
                nc.vector.tensor_copy(out=out_sb[:, 4 * c : 4 * c + 4, :], in_=pso)
                nc.sync.dma_start(
                    out=o_r[:, 4 * c : 4 * c + 4, :],
                    in_=out_sb[:, 4 * c : 4 * c + 4, :],
                )

    nc.compile()
    return nc


def _get_compiled():
    global _COMPILED
    if _COMPILED is None:
        _COMPILED = _build()
    return _COMPILED


def kernel(Q, K, V):
    Q = np.ascontiguousarray(np.asarray(Q, dtype=np.float32))
    K = np.ascontiguousarray(np.asarray(K, dtype=np.float32))
    V = np.ascontiguousarray(np.asarray(V, dtype=np.float32))
    nc = _get_compiled()
    in_maps = [
        {"Q": Q[i * NQS : (i + 1) * NQS], "K": K, "V": V} for i in range(N_CORES)
    ]
    res = run_bass_kernel_spmd(nc, in_maps, list(range(N_CORES)))
    out = np.concatenate([r["out"] for r in res.results], axis=0)
    return out.astype(np.float32)


# revision 4
# speedup vs baseline: 1.1750x; 1.1750x over previous
"""Distributed single-head attention on 8 TRN2 NeuronCores.

softmax(Q @ K.T / sqrt(128)) @ V  with Q,K,V: [8192, 128] fp32.

Strategy: query-parallel. Q rows are sharded 8 ways (1024 queries/core);
K and V are replicated (no collectives). Each core runs flash-attention
style in the "S^T" layout (partitions = keys) so the PV matmul needs no
transpose of the probability tiles:

  S^T[k, q] = (KT_tile).T @ QT          (KT tile stationary, QT moving)
  P^T       = exp(S^T / sqrt(128))      (ACT, fused scale; no max-sub
                                         needed: |scores| <= ~6 in fp32)
  O^T[d, q] += (V_tile).T @ P^T         (V tile is [keys, d] in DRAM =
                                         already the stationary layout)
  l[q]      = colsum(sum_t P^T_t)       (bf16 accum + PE transp/reduce)
  O         = transpose(O^T) * (1/l)

Matmuls in bf16 (fp32 matmul is 4 cyc/row on TRN2; bf16 is 1), fp32
PSUM accumulation. Single sweep over the 64 key tiles with both query
chunks interleaved; K tiles are DMA'd, cast, PE-transposed and consumed
in a software pipeline (PV is emitted 2 key-tiles behind S so the PE
never waits on the exp).
"""

import sys

try:
    import concourse  # noqa: F401
except ImportError:  # grading container fallback
    sys.path.insert(0, "/opt/trn_rl_repo")

import numpy as np

import concourse.bass as bass
import concourse.tile as tile
from concourse import bacc, mybir
from concourse.bass_utils import run_bass_kernel_spmd
from concourse.masks import make_identity

N_CORES = 8
NQ, NK, D = 8192, 8192, 128
NQS = NQ // N_CORES          # queries per core
KT_TILES = NK // 128         # 64 key tiles of 128
QCHUNK = 512                 # matmul moving-dim size (1 PSUM bank fp32)
SCALE = 1.0 / np.sqrt(np.float32(D))
SKEW = 2                     # PV trails S by this many key tiles

F32 = mybir.dt.float32
BF16 = mybir.dt.bfloat16
EXP = mybir.ActivationFunctionType.Exp
COPY = mybir.ActivationFunctionType.Copy

_COMPILED = None


def _build():
    nc = bacc.Bacc(
        "TRN2", target_bir_lowering=False, debug=False, num_devices=N_CORES
    )
    q_d = nc.dram_tensor("Q", [NQS, D], F32, kind="ExternalInput").ap()
    k_d = nc.dram_tensor("K", [NK, D], F32, kind="ExternalInput").ap()
    v_d = nc.dram_tensor("V", [NK, D], F32, kind="ExternalInput").ap()
    o_d = nc.dram_tensor("out", [NQS, D], F32, kind="ExternalOutput").ap()

    # tile views: row = a*128 + p
    q_r = q_d.rearrange("(a p) d -> p a d", p=128)   # [128, 8, 128]
    k_r = k_d.rearrange("(a p) d -> p a d", p=128)   # [128, 64, 128]
    v_r = v_d.rearrange("(a p) d -> p a d", p=128)
    o_r = o_d.rearrange("(a p) d -> p a d", p=128)   # [128, 8, 128]

    with tile.TileContext(nc) as tc:
        with (
            tc.tile_pool(name="persist", bufs=1) as persist,
            tc.tile_pool(name="stage", bufs=3) as stage,
            tc.tile_pool(name="bstage", bufs=3) as bstage,
            tc.tile_pool(name="ktg", bufs=3) as ktgp,
            tc.tile_pool(name="pt", bufs=4) as ptp,
            tc.tile_pool(name="psum_s", bufs=2, space="PSUM") as psum_s,
            tc.tile_pool(name="psum_o", bufs=1, space="PSUM") as psum_o,
            tc.tile_pool(name="psum_tr", bufs=2, space="PSUM") as psum_tr,
        ):
            ident = persist.tile([128, 128], BF16)
            make_identity(nc, ident)

            qt_sb = persist.tile([128, NQS], BF16)     # Q^T  [d, q]
            acc = persist.tile([128, NQS], BF16)       # sum_t P^T tiles
            lq = persist.tile([128, NQS // 128], F32)  # l in [q,1] layout
            rlq = persist.tile([128, NQS // 128], F32)  # 1/l
            out_sb = persist.tile([128, NQS // 128, D], F32)

            nc.gpsimd.memset(acc, 0.0)

            # ---- Q: load, cast, transpose ----
            qst = stage.tile([128, 8, 128], F32, tag="stage")
            nc.sync.dma_start(out=qst, in_=q_r)
            qsb = bstage.tile([128, 8, 128], BF16, tag="bstage")
            nc.vector.tensor_copy(out=qsb, in_=qst)
            for h in range(2):
                ps = psum_tr.tile([128, 512], BF16, tag="tr")
                for j in range(4):
                    nc.tensor.transpose(
                        ps[:, 128 * j : 128 * (j + 1)], qsb[:, 4 * h + j, :], ident
                    )
                nc.vector.tensor_copy(
                    out=qt_sb[:, 512 * h : 512 * (h + 1)], in_=ps
                )

            # ---- main pipeline over 64 key tiles ----
            po = psum_o.tile([128, NQS], F32)  # O^T accum, both chunks
            kt_groups = {}   # 4-tile transposed K groups [d, 512] bf16
            v_stages = {}    # cast V stages [128, 8, 128] bf16
            pts = {}         # exp tiles [128, 1024] bf16 (c0|c1)

            def load_stage(g):  # 8 key tiles per DMA stage
                kst = stage.tile([128, 8, 128], F32, tag="stage")
                nc.sync.dma_start(out=kst, in_=k_r[:, 8 * g : 8 * g + 8, :])
                ksb = bstage.tile([128, 8, 128], BF16, tag="bstage")
                nc.vector.tensor_copy(out=ksb, in_=kst)
                vst = stage.tile([128, 8, 128], F32, tag="stage")
                nc.sync.dma_start(out=vst, in_=v_r[:, 8 * g : 8 * g + 8, :])
                vsb = bstage.tile([128, 8, 128], BF16, tag="vstage")
                nc.vector.tensor_copy(out=vsb, in_=vst)
                v_stages[g] = vsb
                return ksb

            def transpose_group(ksb, h):  # 4 K tiles -> [d, 512] bf16
                ps = psum_tr.tile([128, 512], BF16, tag="tr")
                for j in range(4):
                    nc.tensor.transpose(
                        ps[:, 128 * j : 128 * (j + 1)], ksb[:, 4 * h + j, :], ident
                    )
                ktg = ktgp.tile([128, 512], BF16, tag="ktg")
                nc.vector.tensor_copy(out=ktg, in_=ps)
                return ktg

            def s_exp_add(t):  # S^T matmuls (both chunks), exp, acc add
                ktg = kt_groups[t // 4]
                lhs = ktg[:, 128 * (t % 4) : 128 * (t % 4 + 1)]
                ps = psum_s.tile([128, 1024], F32, tag="ps")
                for c in range(2):
                    nc.tensor.matmul(
                        ps[:, 512 * c : 512 * (c + 1)],
                        lhs,
                        qt_sb[:, 512 * c : 512 * (c + 1)],
                        start=True,
                        stop=True,
                    )
                pt = ptp.tile([128, 1024], BF16, tag="pt")
                nc.scalar.activation(pt, ps, EXP, scale=float(SCALE))
                nc.vector.tensor_add(acc, acc, pt)
                pts[t] = pt

            def pv(t):  # accumulate O^T for both chunks
                pt = pts.pop(t)
                vsb = v_stages[t // 8]
                for c in range(2):
                    nc.tensor.matmul(
                        po[:, 512 * c : 512 * (c + 1)],
                        vsb[:, t % 8, :],
                        pt[:, 512 * c : 512 * (c + 1)],
                        start=(t == 0),
                        stop=(t == KT_TILES - 1),
                    )

            ksb_cur = load_stage(0)
            for t in range(KT_TILES + SKEW):
                if t < KT_TILES:
                    if t % 8 == 0 and t > 0:
                        ksb_cur = load_stage(t // 8)
                    if t % 4 == 0:
                        kt_groups[t // 4] = transpose_group(ksb_cur, (t % 8) // 4)
                    s_exp_add(t)
                if t >= SKEW:
                    pv(t - SKEW)

            # ---- epilogue ----
            for c in range(2):
                qs = slice(512 * c, 512 * (c + 1))
                # l via transpose of acc + free-dim reduce -> [q, 1] layout
                pa = psum_tr.tile([128, 512], BF16, tag="tr")
                for j in range(4):
                    nc.tensor.transpose(
                        pa[:, 128 * j : 128 * (j + 1)],
                        acc[:, 512 * c + 128 * j : 512 * c + 128 * (j + 1)],
                        ident,
                    )
                nc.vector.tensor_reduce(
                    lq[:, 4 * c : 4 * c + 4],
                    pa.rearrange("p (a d) -> p a d", a=4),
                    axis=mybir.AxisListType.X,
                    op=mybir.AluOpType.add,
                )
                # O^T -> bf16 sbuf (ACT; idle by now), transpose, scale
                ob = bstage.tile([128, 512], BF16, tag="ob")
                nc.scalar.activation(ob, po[:, qs], COPY)
                pso = psum_tr.tile([128, 512], BF16, tag="tr")
                for j in range(4):
                    nc.tensor.transpose(
                        pso[:, 128 * j : 128 * (j + 1)],
                        ob[:, 128 * j : 128 * (j + 1)],
                        ident,
                    )
                if c == 0:
                    nc.vector.reciprocal(rlq[:, 0:4], lq[:, 0:4])
                else:
                    nc.vector.reciprocal(rlq[:, 4:8], lq[:, 4:8])
                for j in range(4):
                    a = 4 * c + j
                    nc.vector.tensor_scalar_mul(
                        out_sb[:, a, :],
                        pso[:, 128 * j : 128 * (j + 1)],
                        rlq[:, a : a + 1],
                    )
                nc.sync.dma_start(
                    out=o_r[:, 4 * c : 4 * c + 4, :],
                    in_=out_sb[:, 4 * c : 4 * c + 4, :],
                )

    nc.compile()
    return nc


def _get_compiled():
    global _COMPILED
    if _COMPILED is None:
        _COMPILED = _build()
    return _COMPILED


def kernel(Q, K, V):
    Q = np.ascontiguousarray(np.asarray(Q, dtype=np.float32))
    K = np.ascontiguousarray(np.asarray(K, dtype=np.float32))
    V = np.ascontiguousarray(np.asarray(V, dtype=np.float32))
    nc = _get_compiled()
    in_maps = [
        {"Q": Q[i * NQS : (i + 1) * NQS], "K": K, "V": V} for i in range(N_CORES)
    ]
    res = run_bass_kernel_spmd(nc, in_maps, list(range(N_CORES)))
    out = np.concatenate([r["out"] for r in res.results], axis=0)
    return out.astype(np.float32)
